# revision 18
# baseline (speedup 1.0000x reference)
"""Trainium2 Bass kernel for nn_AggregatorV1 (sparse_attention).

Sharding: data-parallel over batch B=8 across 8 NeuronCores (1 sample/core).
Per core: 2-layer LSTM scan (T=1024) -> LayerNorm -> chunk pivots ->
eof-prob gating -> expected multi-head attention.

Layout conventions (per core):
  - "T-last" buffers: [128 part, (chunk, T)] with t innermost (unit stride)
  - gates permuted host-side to [i, f, o, g] blocks so sigmoid covers cols 0:12
    and tanh cols 12:16 of the per-step gate tile [128, 16].
  - matmuls use bf16 operands, fp32 PSUM accumulation.
"""

import numpy as np
import ml_dtypes

import concourse.bass as bass
import concourse.bacc as bacc
import concourse.mybir as mybir
import concourse.tile as tile
from concourse.bass_utils import run_bass_kernel_spmd

BF16 = mybir.dt.bfloat16
F32 = mybir.dt.float32
F32R = mybir.dt.float16  # fp16: tf32-class precision + FWL speed
AF = mybir.ActivationFunctionType
ALU = mybir.AluOpType

B, T, F, H, FF = 8, 1024, 512, 512, 512
CS = 32
K = T // CS          # 32 chunks
NH = 8               # heads
HD = FF // NH        # 64
NG = 16              # gate m-tiles (2048/128)
NC4 = 4              # hid chunks (512/128)
NT = T // 128        # 8 t-tiles
SC_DP = float(1.0 / np.sqrt(FF))
SC_AT = float(1.0 / np.sqrt(HD))
LN_EPS = 1e-5


def build_program():
    nc = bacc.Bacc(
        "TRN2", target_bir_lowering=False, debug=False, enable_asserts=False
    )

    def din(name, shape, dt=BF16):
        return nc.dram_tensor(name, list(shape), dt, kind="ExternalInput").ap()

    # --- inputs (host-prepped layouts) ---
    xT_d = din("xT", (128, 4 * T), F32)                 # x.T  [p, fc*T+t]
    wihT_d = din("wihT", (2, 128, 4 * 2048), F32)
    whhT_d = din("whhT", (2, 128, 4 * 2048), F32)
    bias_d = din("biascols", (2, 128, NG), F32)         # per-layer gate bias cols
    ident_d = din("ident", (128, 128))
    identf_d = din("identf", (128, 128), F32)
    st_d = din("sel", (128, NT * K))                    # chunk selector S.T
    wcpT_d = din("wcpT", (128, 4 * FF))
    bcp_d = din("bcpcols", (128, 4), F32)
    wffT_d = din("wffT", (128, 4 * FF))
    bff_d = din("bffcols", (128, 4), F32)
    wqT_d = din("wqT", (128, 4 * FF))
    bq_d = din("bqcols", (128, 4), F32)
    wkT_d = din("wkT", (128, 4 * FF))
    bk_d = din("bkcols", (128, 4), F32)
    wvT_d = din("wvT", (128, 4 * FF))
    bvB_d = din("bvB", (128, FF), F32)
    woutT_d = din("woutT", (128, 4 * F))
    boutB_d = din("boutB", (K, F), F32)
    gamB_d = din("gamB", (128, H), F32)
    betB_d = din("betB", (128, H), F32)
    eps_d = din("epscol", (128, 1), F32)
    zh_d = din("zeros4", (128, NC4), F32R)
    noise_d = din("noise", (K, T), F32)
    A_d = din("Amask", (K, T))
    C_d = din("Cmask", (K, T))

    octx_d = nc.dram_tensor("out_ctx", [K, F], F32, kind="ExternalOutput").ap()
    oprb_d = nc.dram_tensor("out_probs", [K, T], F32, kind="ExternalOutput").ap()

    with tile.TileContext(nc) as tc:
        with (
            tc.tile_pool(name="w", bufs=1) as wp,
            tc.tile_pool(name="big", bufs=1) as bigp,
            tc.tile_pool(name="state", bufs=1) as stp,
            tc.tile_pool(name="ps", bufs=1, space="PSUM") as psp,
            tc.tile_pool(name="ps_scan", bufs=1, space="PSUM") as pssc,
            tc.tile_pool(name="small", bufs=1) as smp,
        ):
            # ---------- load everything ----------
            def load(pool, dram, shape, dt=BF16, tag=""):
                t = pool.tile(list(shape), dt, name=f"ld_{dram.name}", tag=tag or dram.name)
                nc.sync.dma_start(t[:], dram)
                return t

            xT = load(bigp, xT_d[:], (128, 4 * T), F32, tag="xv")
            ident = load(wp, ident_d[:], (128, 128))
            identf = load(wp, identf_d[:], (128, 128), F32)
            st_sel = load(wp, st_d[:], (128, NT * K))
            wcpT = load(wp, wcpT_d[:], (128, 4 * FF))
            wffT = load(wp, wffT_d[:], (128, 4 * FF))
            wqT = load(wp, wqT_d[:], (128, 4 * FF))
            wkT = load(wp, wkT_d[:], (128, 4 * FF))
            wvT = load(wp, wvT_d[:], (128, 4 * FF))
            woutT = load(wp, woutT_d[:], (128, 4 * F))
            bcp = load(wp, bcp_d[:], (128, 4), F32)
            bff = load(wp, bff_d[:], (128, 4), F32)
            bq = load(wp, bq_d[:], (128, 4), F32)
            bk = load(wp, bk_d[:], (128, 4), F32)
            bvB = load(wp, bvB_d[:], (128, FF), F32)
            boutB = load(wp, boutB_d[:], (K, F), F32)
            gamB = load(wp, gamB_d[:], (128, H), F32)
            betB = load(wp, betB_d[:], (128, H), F32)
            epsc = load(wp, eps_d[:], (128, 1), F32)
            noise = load(wp, noise_d[:], (K, T), F32)
            Am = load(wp, A_d[:], (K, T))
            Cm = load(wp, C_d[:], (K, T))

            # big working buffers
            xg_hi = bigp.tile([128, NG, T], BF16, name="xg_hi", tag="xgh")
            xg_lo = bigp.tile([128, NG, T], BF16, name="xg_lo")
            h1buf = bigp.tile([128, NC4, T], F32, name="h1buf", tag="hbuf")

            # scan state
            h_st = stp.tile([128, NC4], F32, name="h_st")
            c_st = stp.tile([128, NC4], F32, name="c_st")
            G = stp.tile([128, NG], F32, name="G")
            S = stp.tile([128, NG], F32, name="S")
            Tg = stp.tile([128, 4], F32, name="Tg")
            Tc = stp.tile([128, 4], F32, name="Tc")
            tmp4 = stp.tile([128, 4], F32, name="tmp4")
            pg = pssc.tile([128, NG], F32, name="pg")

            # ---------- per-layer: xg matmul + scan ----------
            for layer in range(2):
                wih = wp.tile([128, 4 * 2048], F32, name=f"wih{layer}", tag="lstmw")
                nc.sync.dma_start(wih[:], wihT_d[layer])
                bcol = wp.tile([128, NG], F32, name=f"bcol{layer}", tag="bcol")
                nc.sync.dma_start(bcol[:], bias_d[layer])

                # xg = input @ WihT + bias   (input: xT for l0, h1buf for l1)
                rhsbuf = xT if layer == 0 else h1buf.rearrange("p c t -> p (c t)")
                for mt in range(NG):
                    for tb in range(2):
                        pxg = psp.tile([128, 512], F32, name="pxg", tag="pA", bufs=2)
                        for kc in range(NC4):
                            nc.tensor.matmul(
                                pxg[:],
                                wih[:, kc * 2048 + mt * 128: kc * 2048 + (mt + 1) * 128].bitcast(F32R),
                                rhsbuf[:, kc * T + tb * 512: kc * T + tb * 512 + 512].bitcast(F32R),
                                start=(kc == 0), stop=(kc == NC4 - 1),
                            )
                        nc.vector.tensor_scalar(
                            xg_hi[:, mt, tb * 512: tb * 512 + 512],
                            pxg[:], bcol[:, mt: mt + 1], None, ALU.add,
                        )
                        nc.vector.scalar_tensor_tensor(
                            xg_lo[:, mt, tb * 512: tb * 512 + 512],
                            pxg[:], bcol[:, mt: mt + 1],
                            xg_hi[:, mt, tb * 512: tb * 512 + 512],
                            ALU.add, ALU.subtract,
                        )
                whh = wp.tile([128, 4 * 2048], F32, name=f"whh{layer}", tag="lstmw")
                nc.sync.dma_start(whh[:], whhT_d[layer])

                # init state
                nc.sync.dma_start(hs2[0][:], zh_d[:])
                nc.vector.memset(c_st[:], 0.0)

                if layer == 0:
                    outbuf = h1buf
                else:
                    h2buf = bigp.tile([128, NC4, T], F32, name="h2buf", tag="hbuf")
                    outbuf = h2buf
                UNROLL = 64
                with tc.For_i(0, T, UNROLL, hint_engines=(mybir.EngineType.PE,), staggered_reset=True) as iv:
                    for u in range(UNROLL):
                        ii = iv + u
                        for mt in range(NG):
                            for kc in range(NC4):
                                nc.tensor.matmul(
                                    pg[:, mt: mt + 1],
                                    whh[:, kc * 2048 + mt * 128: kc * 2048 + (mt + 1) * 128].bitcast(F32R),
                                    h_st[:, kc: kc + 1].bitcast(F32R),
                                    start=(kc == 0), stop=(kc == NC4 - 1),
                                )
                        xg_ht = xg_hi[:, :, bass.ds(ii, 1)].rearrange("p m o -> p (m o)")
                        xg_lt = xg_lo[:, :, bass.ds(ii, 1)].rearrange("p m o -> p (m o)")
                        nc.vector.tensor_add(G[:], pg[:], xg_ht)
                        nc.vector.tensor_add(G[:], G[:], xg_lt)
                        nc.scalar.activation(S[:], G[:, 0:12], AF.Sigmoid)
                        nc.scalar.activation(Tg[:], G[:, 12:16], AF.Tanh)
                        nc.vector.tensor_mul(c_st[:], S[:, 4:8], c_st[:])
                        nc.vector.tensor_mul(tmp4[:], S[:, 0:4], Tg[:])
                        nc.vector.tensor_add(c_st[:], c_st[:], tmp4[:])
                        nc.scalar.activation(Tc[:], c_st[:], AF.Tanh)
                        nc.vector.tensor_mul(h_st[:], S[:, 8:12], Tc[:])
                        nc.vector.tensor_copy(
                            outbuf[:, :, bass.ds(ii, 1)],
                            h_st[:].rearrange("p (c o) -> p c o", o=1),
                        )

            # ---------- LayerNorm (transpose h2buf -> [t, hid], normalize) ----------
            normed = bigp.tile([128, NT, H], BF16, name="normed", tag="nk")   # [t, (tt,hid)]
            ln_in = smp.tile([128, H], F32, name="ln_in")
            ln_s = smp.tile([128, 1], F32, name="ln_s")
            ln_mu = smp.tile([128, 1], F32, name="ln_mu")
            ln_vs = smp.tile([128, 1], F32, name="ln_vs")
            ln_sd = smp.tile([128, 1], F32, name="ln_sd")
            ln_rs = smp.tile([128, 1], F32, name="ln_rs")
            ln_t2 = smp.tile([128, H], F32, name="ln_t2")

            for tt in range(NT):
                for c in range(NC4):
                    ptr = psp.tile([128, 128], F32, name="ptr", tag="pT", bufs=1)
                    nc.tensor.transpose(
                        ptr[:], h2buf[:, c, tt * 128:(tt + 1) * 128], identf[:]
                    )
                    nc.vector.tensor_copy(ln_in[:, c * 128:(c + 1) * 128], ptr[:])
                nc.vector.reduce_sum(ln_s[:], ln_in[:], axis=mybir.AxisListType.X)
                nc.scalar.activation(ln_mu[:], ln_s[:], AF.Copy, scale=1.0 / H)
                nc.vector.tensor_scalar_sub(ln_in[:], ln_in[:], ln_mu[:])
                nc.scalar.activation(ln_t2[:], ln_in[:], AF.Square, accum_out=ln_vs[:])
                nc.scalar.activation(
                    ln_sd[:], ln_vs[:], AF.Sqrt, bias=epsc[:], scale=1.0 / H
                )
                nc.vector.reciprocal(ln_rs[:], ln_sd[:])
                nc.vector.scalar_tensor_tensor(
                    ln_t2[:], ln_in[:], ln_rs[:], gamB[:], ALU.mult, ALU.mult
                )
                nc.vector.tensor_add(normed[:, tt, :], ln_t2[:], betB[:])

            # ---------- normedT [hid, t] ----------
            normedT = bigp.tile([128, NC4, T], BF16, name="normedT", tag="xgh")
            for tt in range(NT):
                for c in range(NC4):
                    ptr2 = psp.tile([128, 128], BF16, name="ptr2", tag="pT", bufs=1)
                    nc.tensor.transpose(
                        ptr2[:], normed[:, tt, c * 128:(c + 1) * 128], ident[:]
                    )
                    nc.vector.tensor_copy(
                        normedT[:, c, tt * 128:(tt + 1) * 128], ptr2[:]
                    )

            # ---------- chunk summary transposed: csT [hid, K] ----------
            csT = smp.tile([128, NC4, K], BF16, name="csT")
            for c in range(NC4):
                pcs = psp.tile([128, K], F32, name="pcs", tag="pS", bufs=2)
                for tt in range(NT):
                    nc.tensor.matmul(
                        pcs[:],
                        normed[:, tt, c * 128:(c + 1) * 128],
                        st_sel[:, tt * K:(tt + 1) * K],
                        start=(tt == 0), stop=(tt == NT - 1),
                    )
                nc.vector.tensor_copy(csT[:, c, :], pcs[:])

            # ---------- pivotsT = WcpT-chain [ff, K], qT [ff, K] ----------
            pT = smp.tile([128, NC4, K], BF16, name="pT")
            for fm in range(4):
                ppv = psp.tile([128, K], F32, name="ppv", tag="pS", bufs=2)
                for hc in range(NC4):
                    nc.tensor.matmul(
                        ppv[:],
                        wcpT[:, hc * FF + fm * 128: hc * FF + (fm + 1) * 128],
                        csT[:, hc, :],
                        start=(hc == 0), stop=(hc == 3),
                    )
                nc.vector.tensor_scalar(
                    pT[:, fm, :], ppv[:], bcp[:, fm: fm + 1], None, ALU.add
                )

            qT = smp.tile([128, NC4, K], BF16, name="qT")
            for qm in range(4):
                pq = psp.tile([128, K], F32, name="pq", tag="pS", bufs=2)
                for fc in range(NC4):
                    nc.tensor.matmul(
                        pq[:],
                        wqT[:, fc * FF + qm * 128: fc * FF + (qm + 1) * 128],
                        pT[:, fc, :],
                        start=(fc == 0), stop=(fc == 3),
                    )
                nc.vector.tensor_scalar(
                    qT[:, qm, :], pq[:], bq[:, qm: qm + 1], None, ALU.add
                )

            # ---------- future frames fft [f, t], kT [kdim, t], v [t, vdim] ----------
            fft = bigp.tile([128, NC4, T], BF16, name="fft", tag="hbuf")
            for fm in range(4):
                for tb in range(2):
                    pff = psp.tile([128, 512], F32, name="pff", tag="pA", bufs=2)
                    for hc in range(NC4):
                        nc.tensor.matmul(
                            pff[:],
                            wffT[:, hc * FF + fm * 128: hc * FF + (fm + 1) * 128],
                            normedT[:, hc, tb * 512: tb * 512 + 512],
                            start=(hc == 0), stop=(hc == 3),
                        )
                    nc.vector.tensor_scalar(
                        fft[:, fm, tb * 512: tb * 512 + 512],
                        pff[:], bff[:, fm: fm + 1], None, ALU.add,
                    )

            kT = bigp.tile([128, NC4, T], BF16, name="kT", tag="nk")
            for km in range(4):
                for tb in range(2):
                    pk = psp.tile([128, 512], F32, name="pk", tag="pA", bufs=2)
                    for fc in range(NC4):
                        nc.tensor.matmul(
                            pk[:],
                            wkT[:, fc * FF + km * 128: fc * FF + (km + 1) * 128],
                            fft[:, fc, tb * 512: tb * 512 + 512],
                            start=(fc == 0), stop=(fc == 3),
                        )
                    nc.vector.tensor_scalar(
                        kT[:, km, tb * 512: tb * 512 + 512],
                        pk[:], bk[:, km: km + 1], None, ALU.add,
                    )

            vbuf = bigp.tile([128, NT, FF], BF16, name="vbuf", tag="xv")      # [t, (tt,vdim)]
            for tt in range(NT):
                pv = psp.tile([128, 512], F32, name="pv", tag="pA", bufs=2)
                for fm in range(4):
                    nc.tensor.matmul(
                        pv[:],
                        fft[:, fm, tt * 128:(tt + 1) * 128],
                        wvT[:, fm * FF:(fm + 1) * FF],
                        start=(fm == 0), stop=(fm == 3),
                    )
                nc.vector.tensor_add(vbuf[:, tt, :], pv[:], bvB[:])

            # ---------- dot products -> eof -> in_futctx ----------
            pdp = psp.tile([32, T], F32, name="pdp", tag="pB", bufs=1)
            for tb in range(2):
                for fc in range(NC4):
                    nc.tensor.matmul(
                        pdp[:, tb * 512: tb * 512 + 512],
                        pT[:, fc, :],
                        fft[:, fc, tb * 512: tb * 512 + 512],
                        start=(fc == 0), stop=(fc == 3),
                    )
            eof = smp.tile([K, T], F32, name="eof")
            nc.vector.scalar_tensor_tensor(
                eof[:], pdp[:], SC_DP, noise[:], ALU.mult, ALU.add
            )
            nc.scalar.activation(eof[:], eof[:], AF.Sigmoid)
            nc.vector.tensor_mul(eof[:], eof[:], Am[:])
            nc.vector.tensor_add(eof[:], eof[:], Cm[:])
            infut = smp.tile([K, T], F32, name="infut")
            nc.vector.memset(infut[:, 0:1], 1.0)
            nc.scalar.activation(
                infut[:, 1:T], eof[:, 0: T - 1], AF.Copy, bias=1.0, scale=-1.0
            )
            nc.sync.dma_start(oprb_d, infut[:])

            # replicate infut across 4 head-groups of partitions
            infutB = smp.tile([128, T], BF16, name="infutB")
            for hh in range(4):
                nc.gpsimd.dma_start(infutB[hh * K:(hh + 1) * K, :], infut[:])

            # ---------- attention scores + masked softmax (2 groups of 4 heads) ----
            attnT = smp.tile([128, NT, 256], BF16, name="attnT")  # [t,(tt,grp,hk)]
            for grp in range(2):
                psc = psp.tile([128, T], F32, name="psc", tag="pB", bufs=1)
                for hh in range(4):
                    h = grp * 4 + hh
                    qm, qr = h // 2, (h % 2) * 64
                    for tb in range(2):
                        nc.tensor.matmul(
                            psc[hh * 32:(hh + 1) * 32, tb * 512: tb * 512 + 512],
                            qT[qr: qr + 64, qm, :],
                            kT[qr: qr + 64, qm, tb * 512: tb * 512 + 512],
                            start=True, stop=True,
                            tile_position=(qr, hh * 32),
                        )
                smx = smp.tile([128, 1], F32, name="smx", tag="smx")
                nc.vector.reduce_max(smx[:], psc[:], axis=mybir.AxisListType.X)
                sbm = smp.tile([128, 1], F32, name="sbm", tag="sbm")
                nc.vector.tensor_scalar_mul(sbm[:], smx[:], -SC_AT)
                sme = smp.tile([128, T], BF16, name="sme", tag="sme")
                nc.scalar.activation(sme[:], psc[:], AF.Exp, bias=sbm[:], scale=SC_AT)
                smden = smp.tile([128, 1], F32, name="smden", tag="smden")
                nc.vector.scalar_tensor_tensor(
                    sme[:], sme[:], 1.0, infutB[:], ALU.mult, ALU.mult,
                    accum_out=smden[:],
                )
                smw = sme
                smr = smp.tile([128, 1], F32, name="smr", tag="smr")
                nc.vector.reciprocal(smr[:], smden[:])
                attn = smp.tile([128, T], BF16, name="attn", tag="attn")
                nc.vector.tensor_scalar_mul(attn[:], smw[:], smr[:])
                for tt in range(NT):
                    pat = psp.tile([128, 128], BF16, name="pat", tag="pT", bufs=1)
                    nc.tensor.transpose(
                        pat[:], attn[:, tt * 128:(tt + 1) * 128], ident[:]
                    )
                    nc.vector.tensor_copy(
                        attnT[:, tt, grp * 128:(grp + 1) * 128], pat[:]
                    )

            # ---------- ctx = attn @ v  (per head), out projection ----------
            pctx = psp.tile([32, FF], F32, name="pctx", tag="pS", bufs=2)
            for h in range(NH):
                grp, hh = h // 4, h % 4
                for tt in range(NT):
                    nc.tensor.matmul(
                        pctx[:, h * HD:(h + 1) * HD],
                        attnT[:, tt, grp * 128 + hh * 32: grp * 128 + (hh + 1) * 32],
                        vbuf[:, tt, h * HD:(h + 1) * HD],
                        start=(tt == 0), stop=(tt == NT - 1),
                    )
            ctx_sb = smp.tile([K, FF], BF16, name="ctx_sb")
            nc.vector.tensor_copy(ctx_sb[:], pctx[:])
            ctxT = smp.tile([128, 4, K], BF16, name="ctxT")
            for q in range(4):
                pct = psp.tile([128, K], BF16, name="pct", tag="pS", bufs=2)
                nc.tensor.transpose(
                    pct[:], ctx_sb[:, q * 128:(q + 1) * 128], ident[0:K, 0:K]
                )
                nc.vector.tensor_copy(ctxT[:, q, :], pct[:])

            po = psp.tile([K, F], F32, name="po", tag="pS", bufs=2)
            for q in range(4):
                nc.tensor.matmul(
                    po[:], ctxT[:, q, :], woutT[:, q * F:(q + 1) * F],
                    start=(q == 0), stop=(q == 3),
                )
            octx = smp.tile([K, F], F32, name="octx")
            nc.vector.tensor_add(octx[:], po[:], boutB[:])
            nc.sync.dma_start(octx_d, octx[:])

    nc.compile()
    return nc


# ---------------------------------------------------------------------------
# host-side prep
# ---------------------------------------------------------------------------
def _bf(a):
    return np.ascontiguousarray(a.astype(ml_dtypes.bfloat16))


def _f32(a):
    return np.ascontiguousarray(a.astype(np.float32))


def _f16(a):
    return np.ascontiguousarray(a.astype(np.float16))


def _chunked_cols(wT):
    """[D, M] -> [128, (D//128)*M] with row = d%128, block-major d//128."""
    D, M = wT.shape
    return np.ascontiguousarray(
        wT.reshape(D // 128, 128, M).transpose(1, 0, 2).reshape(128, -1)
    )


def prepare_in_maps(inputs):
    x = np.asarray(inputs["input"], np.float32)
    mask = np.asarray(inputs["sequence_mask"]).astype(np.float32)
    noise = np.asarray(inputs["noise"], np.float32)
    Wih = np.asarray(inputs["lstm_Wih"], np.float32)
    Whh = np.asarray(inputs["lstm_Whh"], np.float32)
    bih = np.asarray(inputs["lstm_bih"], np.float32)
    bhh = np.asarray(inputs["lstm_bhh"], np.float32)
    gam = np.asarray(inputs["ln_gamma"], np.float32)
    bet = np.asarray(inputs["ln_beta"], np.float32)
    Wcp = np.asarray(inputs["Wcp"], np.float32)
    bcp = np.asarray(inputs["bcp"], np.float32)
    Wff = np.asarray(inputs["Wff"], np.float32)
    bff = np.asarray(inputs["bff"], np.float32)
    Wq = np.asarray(inputs["Wq"], np.float32)
    bq = np.asarray(inputs["bq"], np.float32)
    Wkv = np.asarray(inputs["Wkv"], np.float32)
    bkv = np.asarray(inputs["bkv"], np.float32)
    Wout = np.asarray(inputs["Wout"], np.float32)
    bout = np.asarray(inputs["bout"], np.float32)
    assert int(inputs["chunk_size"]) == CS

    perm = np.concatenate([
        np.arange(0, 512), np.arange(512, 1024),
        np.arange(1536, 2048), np.arange(1024, 1536),
    ])  # [i, f, o, g]

    gdbl = np.ones((2048, 1), np.float32)
    gdbl[1536:] = 2.0  # g-gates doubled: tanh(x) = 2*sigmoid(2x) - 1
    wihT = np.stack([_chunked_cols((Wih[l][perm] * gdbl).T) for l in range(2)])
    whhT = np.stack([_chunked_cols((Whh[l][perm] * gdbl).T) for l in range(2)])
    biascols = np.stack([
        ((bih[l] + bhh[l])[perm] * gdbl[:, 0]).reshape(NG, 128).T
        for l in range(2)
    ])

    st = np.zeros((128, NT * K), np.float32)
    for tt in range(NT):
        for p in range(128):
            st[p, tt * K + (tt * 128 + p) // CS] = 1.0

    winkeep = (np.arange(T)[None, :] >= (np.arange(K)[:, None] + 1) * CS)

    common = {
        "wihT": _f16(wihT), "whhT": _f16(whhT), "biascols": _f32(biascols),
        "ident": _bf(np.eye(128)), "identf": _f16(np.eye(128)), "sel": _bf(st),
        "wcpT": _bf(_chunked_cols(Wcp.T)), "bcpcols": _f32(bcp.reshape(4, 128).T),
        "wffT": _bf(_chunked_cols(Wff.T)), "bffcols": _f32(bff.reshape(4, 128).T),
        "wqT": _bf(_chunked_cols(Wq.T)), "bqcols": _f32(bq.reshape(4, 128).T),
        "wkT": _bf(_chunked_cols(Wkv[:FF].T)),
        "bkcols": _f32(bkv[:FF].reshape(4, 128).T),
        "wvT": _bf(_chunked_cols(Wkv[FF:].T)),
        "bvB": _f32(np.broadcast_to(bkv[FF:], (128, FF))),
        "woutT": _bf(_chunked_cols(Wout.T)),
        "boutB": _f32(np.broadcast_to(bout, (K, F))),
        "gamB": _f32(np.broadcast_to(gam, (128, H))),
        "betB": _f32(np.broadcast_to(bet, (128, H))),
        "epscol": _f32(np.full((128, 1), LN_EPS)),
        "zeros4": _f16(np.zeros((128, NC4))),
    }

    in_maps = []
    for b in range(B):
        m = dict(common)
        m["xT"] = _f16(_chunked_cols(x[b].T))
        m["noise"] = _f32(noise[b])
        m["Amask"] = _bf(winkeep * mask[b][None, :])
        m["Cmask"] = _bf(1.0 - mask[b][None, :] * np.ones((K, T), np.float32))
        in_maps.append(m)
    return in_maps


_NC_CACHE = {}


def kernel(**inputs):
    if "nc" not in _NC_CACHE:
        _NC_CACHE["nc"] = build_program()
    nc = _NC_CACHE["nc"]
    in_maps = prepare_in_maps(inputs)
    res = run_bass_kernel_spmd(nc, in_maps, core_ids=list(range(B)))
    ctx = np.stack([res.results[b]["out_ctx"] for b in range(B)])
    prb = np.stack([res.results[b]["out_probs"] for b in range(B)])
    return np.asarray(ctx, np.float32), np.asarray(prb, np.float32)


# revision 20
# speedup vs baseline: 1.1964x; 1.1964x over previous
"""Trainium2 Bass kernel for nn_AggregatorV1 (sparse_attention).

Sharding: data-parallel over batch B=8 across 8 NeuronCores (1 sample/core).
Per core: 2-layer LSTM scan (T=1024) -> LayerNorm -> chunk pivots ->
eof-prob gating -> expected multi-head attention.

Layout conventions (per core):
  - "T-last" buffers: [128 part, (chunk, T)] with t innermost (unit stride)
  - gates permuted host-side to [i, f, o, g] blocks so sigmoid covers cols 0:12
    and tanh cols 12:16 of the per-step gate tile [128, 16].
  - matmuls use bf16 operands, fp32 PSUM accumulation.
"""

import numpy as np
import ml_dtypes

import concourse.bass as bass
import concourse.bacc as bacc
import concourse.mybir as mybir
import concourse.tile as tile
from concourse.bass_utils import run_bass_kernel_spmd

BF16 = mybir.dt.bfloat16
F32 = mybir.dt.float32
F32R = mybir.dt.float16  # fp16: tf32-class precision + FWL speed
AF = mybir.ActivationFunctionType
ALU = mybir.AluOpType

B, T, F, H, FF = 8, 1024, 512, 512, 512
CS = 32
K = T // CS          # 32 chunks
NH = 8               # heads
HD = FF // NH        # 64
NG = 16              # gate m-tiles (2048/128)
NC4 = 4              # hid chunks (512/128)
NT = T // 128        # 8 t-tiles
SC_DP = float(1.0 / np.sqrt(FF))
SC_AT = float(1.0 / np.sqrt(HD))
LN_EPS = 1e-5


def build_program():
    nc = bacc.Bacc(
        "TRN2", target_bir_lowering=False, debug=False, enable_asserts=False
    )

    def din(name, shape, dt=BF16):
        return nc.dram_tensor(name, list(shape), dt, kind="ExternalInput").ap()

    # --- inputs (host-prepped layouts) ---
    xT_d = din("xT", (128, 4 * T), F32)                 # x.T  [p, fc*T+t]
    wihT_d = din("wihT", (2, 128, 4 * 2048), F32)
    whhT_d = din("whhT", (2, 128, 4 * 2048), F32)
    bias_d = din("biascols", (2, 128, NG), F32)         # per-layer gate bias cols
    ident_d = din("ident", (128, 128))
    identf_d = din("identf", (128, 128), F32)
    st_d = din("sel", (128, NT * K))                    # chunk selector S.T
    wcpT_d = din("wcpT", (128, 4 * FF))
    bcp_d = din("bcpcols", (128, 4), F32)
    wffT_d = din("wffT", (128, 4 * FF))
    bff_d = din("bffcols", (128, 4), F32)
    wqT_d = din("wqT", (128, 4 * FF))
    bq_d = din("bqcols", (128, 4), F32)
    wkT_d = din("wkT", (128, 4 * FF))
    bk_d = din("bkcols", (128, 4), F32)
    wvT_d = din("wvT", (128, 4 * FF))
    bvB_d = din("bvB", (128, FF), F32)
    woutT_d = din("woutT", (128, 4 * F))
    boutB_d = din("boutB", (K, F), F32)
    gamB_d = din("gamB", (128, H), F32)
    betB_d = din("betB", (128, H), F32)
    eps_d = din("epscol", (128, 1), F32)
    zh_d = din("zeros4", (128, NC4), F32R)
    noise_d = din("noise", (K, T), F32)
    A_d = din("Amask", (K, T))
    C_d = din("Cmask", (K, T))

    octx_d = nc.dram_tensor("out_ctx", [K, F], F32, kind="ExternalOutput").ap()
    oprb_d = nc.dram_tensor("out_probs", [K, T], F32, kind="ExternalOutput").ap()

    with tile.TileContext(nc) as tc:
        with (
            tc.tile_pool(name="w", bufs=1) as wp,
            tc.tile_pool(name="big", bufs=1) as bigp,
            tc.tile_pool(name="state", bufs=1) as stp,
            tc.tile_pool(name="ps", bufs=1, space="PSUM") as psp,
            tc.tile_pool(name="ps_scan", bufs=1, space="PSUM") as pssc,
            tc.tile_pool(name="small", bufs=1) as smp,
        ):
            # ---------- load everything ----------
            def load(pool, dram, shape, dt=BF16, tag=""):
                t = pool.tile(list(shape), dt, name=f"ld_{dram.name}", tag=tag or dram.name)
                nc.sync.dma_start(t[:], dram)
                return t

            xT = load(bigp, xT_d[:], (128, 4 * T), F32, tag="xv")
            ident = load(wp, ident_d[:], (128, 128))
            identf = load(wp, identf_d[:], (128, 128), F32)
            st_sel = load(wp, st_d[:], (128, NT * K))
            wcpT = load(wp, wcpT_d[:], (128, 4 * FF))
            wffT = load(wp, wffT_d[:], (128, 4 * FF))
            wqT = load(wp, wqT_d[:], (128, 4 * FF))
            wkT = load(wp, wkT_d[:], (128, 4 * FF))
            wvT = load(wp, wvT_d[:], (128, 4 * FF))
            woutT = load(wp, woutT_d[:], (128, 4 * F))
            bcp = load(wp, bcp_d[:], (128, 4), F32)
            bff = load(wp, bff_d[:], (128, 4), F32)
            bq = load(wp, bq_d[:], (128, 4), F32)
            bk = load(wp, bk_d[:], (128, 4), F32)
            bvB = load(wp, bvB_d[:], (128, FF), F32)
            boutB = load(wp, boutB_d[:], (K, F), F32)
            gamB = load(wp, gamB_d[:], (128, H), F32)
            betB = load(wp, betB_d[:], (128, H), F32)
            epsc = load(wp, eps_d[:], (128, 1), F32)
            noise = load(wp, noise_d[:], (K, T), F32)
            Am = load(wp, A_d[:], (K, T))
            Cm = load(wp, C_d[:], (K, T))

            # big working buffers
            xg_hi = bigp.tile([128, NG, T], BF16, name="xg_hi", tag="xgh")
            xg_lo = bigp.tile([128, NG, T], BF16, name="xg_lo")
            h1buf = bigp.tile([128, NC4, T], F32, name="h1buf", tag="hbuf")

            # scan state
            h_st = stp.tile([128, NC4], F32, name="h_st")
            c_st = stp.tile([128, NC4], F32, name="c_st")
            G = stp.tile([128, NG], F32, name="G")
            S = stp.tile([128, NG], F32, name="S")
            Tg = stp.tile([128, 4], F32, name="Tg")
            Tc = stp.tile([128, 4], F32, name="Tc")
            tmp4 = stp.tile([128, 4], F32, name="tmp4")
            pg = pssc.tile([128, NG], F32, name="pg")

            # ---------- per-layer: xg matmul + scan ----------
            for layer in range(2):
                wih = wp.tile([128, 4 * 2048], F32, name=f"wih{layer}", tag="lstmw")
                nc.sync.dma_start(wih[:], wihT_d[layer])
                bcol = wp.tile([128, NG], F32, name=f"bcol{layer}", tag="bcol")
                nc.sync.dma_start(bcol[:], bias_d[layer])

                # xg = input @ WihT + bias   (input: xT for l0, h1buf for l1)
                rhsbuf = xT if layer == 0 else h1buf.rearrange("p c t -> p (c t)")
                for mt in range(NG):
                    for tb in range(2):
                        pxg = psp.tile([128, 512], F32, name="pxg", tag="pA", bufs=2)
                        for kc in range(NC4):
                            nc.tensor.matmul(
                                pxg[:],
                                wih[:, kc * 2048 + mt * 128: kc * 2048 + (mt + 1) * 128].bitcast(F32R),
                                rhsbuf[:, kc * T + tb * 512: kc * T + tb * 512 + 512].bitcast(F32R),
                                start=(kc == 0), stop=(kc == NC4 - 1),
                            )
                        nc.vector.tensor_scalar(
                            xg_hi[:, mt, tb * 512: tb * 512 + 512],
                            pxg[:], bcol[:, mt: mt + 1], None, ALU.add,
                        )
                        nc.vector.scalar_tensor_tensor(
                            xg_lo[:, mt, tb * 512: tb * 512 + 512],
                            pxg[:], bcol[:, mt: mt + 1],
                            xg_hi[:, mt, tb * 512: tb * 512 + 512],
                            ALU.add, ALU.subtract,
                        )
                whh = wp.tile([128, 4 * 2048], F32, name=f"whh{layer}", tag="lstmw")
                nc.sync.dma_start(whh[:], whhT_d[layer])

                # init state
                nc.sync.dma_start(h_st[:], zh_d[:])
                nc.vector.memset(c_st[:], 0.0)

                if layer == 0:
                    outbuf = h1buf
                else:
                    h2buf = bigp.tile([128, NC4, T], F32, name="h2buf", tag="hbuf")
                    outbuf = h2buf
                UNROLL = 64
                with tc.For_i(0, T, UNROLL, hint_engines=(mybir.EngineType.PE,), staggered_reset=True) as iv:
                    for u in range(UNROLL):
                        ii = iv + u
                        for mt in range(NG):
                            for kc in range(NC4):
                                nc.tensor.matmul(
                                    pg[:, mt: mt + 1],
                                    whh[:, kc * 2048 + mt * 128: kc * 2048 + (mt + 1) * 128].bitcast(F32R),
                                    h_st[:, kc: kc + 1].bitcast(F32R),
                                    start=(kc == 0), stop=(kc == NC4 - 1),
                                )
                        xg_ht = xg_hi[:, :, bass.ds(ii, 1)].rearrange("p m o -> p (m o)")
                        xg_lt = xg_lo[:, :, bass.ds(ii, 1)].rearrange("p m o -> p (m o)")
                        nc.vector.tensor_add(G[:], pg[:], xg_ht)
                        nc.vector.tensor_add(G[:], G[:], xg_lt)
                        nc.scalar.activation(S[:], G[:, 0:12], AF.Sigmoid)
                        nc.scalar.activation(Tg[:], G[:, 12:16], AF.Tanh)
                        nc.vector.tensor_mul(c_st[:], S[:, 4:8], c_st[:])
                        nc.vector.tensor_mul(tmp4[:], S[:, 0:4], Tg[:])
                        nc.vector.tensor_add(c_st[:], c_st[:], tmp4[:])
                        nc.scalar.activation(Tc[:], c_st[:], AF.Tanh)
                        nc.vector.tensor_mul(h_st[:], S[:, 8:12], Tc[:])
                        nc.vector.tensor_copy(
                            outbuf[:, :, bass.ds(ii, 1)],
                            h_st[:].rearrange("p (c o) -> p c o", o=1),
                        )

            # ---------- LayerNorm (transpose h2buf -> [t, hid], normalize) ----------
            normed = bigp.tile([128, NT, H], BF16, name="normed", tag="nk")   # [t, (tt,hid)]
            ln_in = smp.tile([128, H], F32, name="ln_in")
            ln_s = smp.tile([128, 1], F32, name="ln_s")
            ln_mu = smp.tile([128, 1], F32, name="ln_mu")
            ln_vs = smp.tile([128, 1], F32, name="ln_vs")
            ln_sd = smp.tile([128, 1], F32, name="ln_sd")
            ln_rs = smp.tile([128, 1], F32, name="ln_rs")
            ln_t2 = smp.tile([128, H], F32, name="ln_t2")

            for tt in range(NT):
                for c in range(NC4):
                    ptr = psp.tile([128, 128], F32, name="ptr", tag="pT", bufs=1)
                    nc.tensor.transpose(
                        ptr[:], h2buf[:, c, tt * 128:(tt + 1) * 128], identf[:]
                    )
                    nc.vector.tensor_copy(ln_in[:, c * 128:(c + 1) * 128], ptr[:])
                nc.vector.reduce_sum(ln_s[:], ln_in[:], axis=mybir.AxisListType.X)
                nc.scalar.activation(ln_mu[:], ln_s[:], AF.Copy, scale=1.0 / H)
                nc.vector.tensor_scalar_sub(ln_in[:], ln_in[:], ln_mu[:])
                nc.scalar.activation(ln_t2[:], ln_in[:], AF.Square, accum_out=ln_vs[:])
                nc.scalar.activation(
                    ln_sd[:], ln_vs[:], AF.Sqrt, bias=epsc[:], scale=1.0 / H
                )
                nc.vector.reciprocal(ln_rs[:], ln_sd[:])
                nc.vector.scalar_tensor_tensor(
                    ln_t2[:], ln_in[:], ln_rs[:], gamB[:], ALU.mult, ALU.mult
                )
                nc.vector.tensor_add(normed[:, tt, :], ln_t2[:], betB[:])

            # ---------- normedT [hid, t] ----------
            normedT = bigp.tile([128, NC4, T], BF16, name="normedT", tag="xgh")
            for tt in range(NT):
                for c in range(NC4):
                    ptr2 = psp.tile([128, 128], BF16, name="ptr2", tag="pT", bufs=1)
                    nc.tensor.transpose(
                        ptr2[:], normed[:, tt, c * 128:(c + 1) * 128], ident[:]
                    )
                    nc.vector.tensor_copy(
                        normedT[:, c, tt * 128:(tt + 1) * 128], ptr2[:]
                    )

            # ---------- chunk summary transposed: csT [hid, K] ----------
            csT = smp.tile([128, NC4, K], BF16, name="csT")
            for c in range(NC4):
                pcs = psp.tile([128, K], F32, name="pcs", tag="pS", bufs=2)
                for tt in range(NT):
                    nc.tensor.matmul(
                        pcs[:],
                        normed[:, tt, c * 128:(c + 1) * 128],
                        st_sel[:, tt * K:(tt + 1) * K],
                        start=(tt == 0), stop=(tt == NT - 1),
                    )
                nc.vector.tensor_copy(csT[:, c, :], pcs[:])

            # ---------- pivotsT = WcpT-chain [ff, K], qT [ff, K] ----------
            pT = smp.tile([128, NC4, K], BF16, name="pT")
            for fm in range(4):
                ppv = psp.tile([128, K], F32, name="ppv", tag="pS", bufs=2)
                for hc in range(NC4):
                    nc.tensor.matmul(
                        ppv[:],
                        wcpT[:, hc * FF + fm * 128: hc * FF + (fm + 1) * 128],
                        csT[:, hc, :],
                        start=(hc == 0), stop=(hc == 3),
                    )
                nc.vector.tensor_scalar(
                    pT[:, fm, :], ppv[:], bcp[:, fm: fm + 1], None, ALU.add
                )

            qT = smp.tile([128, NC4, K], BF16, name="qT")
            for qm in range(4):
                pq = psp.tile([128, K], F32, name="pq", tag="pS", bufs=2)
                for fc in range(NC4):
                    nc.tensor.matmul(
                        pq[:],
                        wqT[:, fc * FF + qm * 128: fc * FF + (qm + 1) * 128],
                        pT[:, fc, :],
                        start=(fc == 0), stop=(fc == 3),
                    )
                nc.vector.tensor_scalar(
                    qT[:, qm, :], pq[:], bq[:, qm: qm + 1], None, ALU.add
                )

            # ---------- future frames fft [f, t], kT [kdim, t], v [t, vdim] ----------
            fft = bigp.tile([128, NC4, T], BF16, name="fft", tag="hbuf")
            for fm in range(4):
                for tb in range(2):
                    pff = psp.tile([128, 512], F32, name="pff", tag="pA", bufs=2)
                    for hc in range(NC4):
                        nc.tensor.matmul(
                            pff[:],
                            wffT[:, hc * FF + fm * 128: hc * FF + (fm + 1) * 128],
                            normedT[:, hc, tb * 512: tb * 512 + 512],
                            start=(hc == 0), stop=(hc == 3),
                        )
                    nc.vector.tensor_scalar(
                        fft[:, fm, tb * 512: tb * 512 + 512],
                        pff[:], bff[:, fm: fm + 1], None, ALU.add,
                    )

            kT = bigp.tile([128, NC4, T], BF16, name="kT", tag="nk")
            for km in range(4):
                for tb in range(2):
                    pk = psp.tile([128, 512], F32, name="pk", tag="pA", bufs=2)
                    for fc in range(NC4):
                        nc.tensor.matmul(
                            pk[:],
                            wkT[:, fc * FF + km * 128: fc * FF + (km + 1) * 128],
                            fft[:, fc, tb * 512: tb * 512 + 512],
                            start=(fc == 0), stop=(fc == 3),
                        )
                    nc.vector.tensor_scalar(
                        kT[:, km, tb * 512: tb * 512 + 512],
                        pk[:], bk[:, km: km + 1], None, ALU.add,
                    )

            vbuf = bigp.tile([128, NT, FF], BF16, name="vbuf", tag="xv")      # [t, (tt,vdim)]
            for tt in range(NT):
                pv = psp.tile([128, 512], F32, name="pv", tag="pA", bufs=2)
                for fm in range(4):
                    nc.tensor.matmul(
                        pv[:],
                        fft[:, fm, tt * 128:(tt + 1) * 128],
                        wvT[:, fm * FF:(fm + 1) * FF],
                        start=(fm == 0), stop=(fm == 3),
                    )
                nc.vector.tensor_add(vbuf[:, tt, :], pv[:], bvB[:])

            # ---------- dot products -> eof -> in_futctx ----------
            pdp = psp.tile([32, T], F32, name="pdp", tag="pB", bufs=1)
            for tb in range(2):
                for fc in range(NC4):
                    nc.tensor.matmul(
                        pdp[:, tb * 512: tb * 512 + 512],
                        pT[:, fc, :],
                        fft[:, fc, tb * 512: tb * 512 + 512],
                        start=(fc == 0), stop=(fc == 3),
                    )
            eof = smp.tile([K, T], F32, name="eof")
            nc.vector.scalar_tensor_tensor(
                eof[:], pdp[:], SC_DP, noise[:], ALU.mult, ALU.add
            )
            nc.scalar.activation(eof[:], eof[:], AF.Sigmoid)
            nc.vector.tensor_mul(eof[:], eof[:], Am[:])
            nc.vector.tensor_add(eof[:], eof[:], Cm[:])
            infut = smp.tile([K, T], F32, name="infut")
            nc.vector.memset(infut[:, 0:1], 1.0)
            nc.scalar.activation(
                infut[:, 1:T], eof[:, 0: T - 1], AF.Copy, bias=1.0, scale=-1.0
            )
            nc.sync.dma_start(oprb_d, infut[:])

            # replicate infut across 4 head-groups of partitions
            infutB = smp.tile([128, T], BF16, name="infutB")
            for hh in range(4):
                nc.gpsimd.dma_start(infutB[hh * K:(hh + 1) * K, :], infut[:])

            # ---------- attention scores + masked softmax (2 groups of 4 heads) ----
            attnT = smp.tile([128, NT, 256], BF16, name="attnT")  # [t,(tt,grp,hk)]
            for grp in range(2):
                psc = psp.tile([128, T], F32, name="psc", tag="pB", bufs=1)
                for hh in range(4):
                    h = grp * 4 + hh
                    qm, qr = h // 2, (h % 2) * 64
                    for tb in range(2):
                        nc.tensor.matmul(
                            psc[hh * 32:(hh + 1) * 32, tb * 512: tb * 512 + 512],
                            qT[qr: qr + 64, qm, :],
                            kT[qr: qr + 64, qm, tb * 512: tb * 512 + 512],
                            start=True, stop=True,
                            tile_position=(qr, hh * 32),
                        )
                smx = smp.tile([128, 1], F32, name="smx", tag="smx")
                nc.vector.reduce_max(smx[:], psc[:], axis=mybir.AxisListType.X)
                sbm = smp.tile([128, 1], F32, name="sbm", tag="sbm")
                nc.vector.tensor_scalar_mul(sbm[:], smx[:], -SC_AT)
                sme = smp.tile([128, T], BF16, name="sme", tag="sme")
                nc.scalar.activation(sme[:], psc[:], AF.Exp, bias=sbm[:], scale=SC_AT)
                smden = smp.tile([128, 1], F32, name="smden", tag="smden")
                nc.vector.scalar_tensor_tensor(
                    sme[:], sme[:], 1.0, infutB[:], ALU.mult, ALU.mult,
                    accum_out=smden[:],
                )
                smw = sme
                smr = smp.tile([128, 1], F32, name="smr", tag="smr")
                nc.vector.reciprocal(smr[:], smden[:])
                attn = smp.tile([128, T], BF16, name="attn", tag="attn")
                nc.vector.tensor_scalar_mul(attn[:], smw[:], smr[:])
                for tt in range(NT):
                    pat = psp.tile([128, 128], BF16, name="pat", tag="pT", bufs=1)
                    nc.tensor.transpose(
                        pat[:], attn[:, tt * 128:(tt + 1) * 128], ident[:]
                    )
                    nc.vector.tensor_copy(
                        attnT[:, tt, grp * 128:(grp + 1) * 128], pat[:]
                    )

            # ---------- ctx = attn @ v  (per head), out projection ----------
            pctx = psp.tile([32, FF], F32, name="pctx", tag="pS", bufs=2)
            for h in range(NH):
                grp, hh = h // 4, h % 4
                for tt in range(NT):
                    nc.tensor.matmul(
                        pctx[:, h * HD:(h + 1) * HD],
                        attnT[:, tt, grp * 128 + hh * 32: grp * 128 + (hh + 1) * 32],
                        vbuf[:, tt, h * HD:(h + 1) * HD],
                        start=(tt == 0), stop=(tt == NT - 1),
                    )
            ctx_sb = smp.tile([K, FF], BF16, name="ctx_sb")
            nc.vector.tensor_copy(ctx_sb[:], pctx[:])
            ctxT = smp.tile([128, 4, K], BF16, name="ctxT")
            for q in range(4):
                pct = psp.tile([128, K], BF16, name="pct", tag="pS", bufs=2)
                nc.tensor.transpose(
                    pct[:], ctx_sb[:, q * 128:(q + 1) * 128], ident[0:K, 0:K]
                )
                nc.vector.tensor_copy(ctxT[:, q, :], pct[:])

            po = psp.tile([K, F], F32, name="po", tag="pS", bufs=2)
            for q in range(4):
                nc.tensor.matmul(
                    po[:], ctxT[:, q, :], woutT[:, q * F:(q + 1) * F],
                    start=(q == 0), stop=(q == 3),
                )
            octx = smp.tile([K, F], F32, name="octx")
            nc.vector.tensor_add(octx[:], po[:], boutB[:])
            nc.sync.dma_start(octx_d, octx[:])

    nc.compile()
    return nc


# ---------------------------------------------------------------------------
# host-side prep
# ---------------------------------------------------------------------------
def _bf(a):
    return np.ascontiguousarray(a.astype(ml_dtypes.bfloat16))


def _f32(a):
    return np.ascontiguousarray(a.astype(np.float32))


def _f16(a):
    return np.ascontiguousarray(a.astype(np.float16))


def _chunked_cols(wT):
    """[D, M] -> [128, (D//128)*M] with row = d%128, block-major d//128."""
    D, M = wT.shape
    return np.ascontiguousarray(
        wT.reshape(D // 128, 128, M).transpose(1, 0, 2).reshape(128, -1)
    )


def prepare_in_maps(inputs):
    x = np.asarray(inputs["input"], np.float32)
    mask = np.asarray(inputs["sequence_mask"]).astype(np.float32)
    noise = np.asarray(inputs["noise"], np.float32)
    Wih = np.asarray(inputs["lstm_Wih"], np.float32)
    Whh = np.asarray(inputs["lstm_Whh"], np.float32)
    bih = np.asarray(inputs["lstm_bih"], np.float32)
    bhh = np.asarray(inputs["lstm_bhh"], np.float32)
    gam = np.asarray(inputs["ln_gamma"], np.float32)
    bet = np.asarray(inputs["ln_beta"], np.float32)
    Wcp = np.asarray(inputs["Wcp"], np.float32)
    bcp = np.asarray(inputs["bcp"], np.float32)
    Wff = np.asarray(inputs["Wff"], np.float32)
    bff = np.asarray(inputs["bff"], np.float32)
    Wq = np.asarray(inputs["Wq"], np.float32)
    bq = np.asarray(inputs["bq"], np.float32)
    Wkv = np.asarray(inputs["Wkv"], np.float32)
    bkv = np.asarray(inputs["bkv"], np.float32)
    Wout = np.asarray(inputs["Wout"], np.float32)
    bout = np.asarray(inputs["bout"], np.float32)
    assert int(inputs["chunk_size"]) == CS

    perm = np.concatenate([
        np.arange(0, 512), np.arange(512, 1024),
        np.arange(1536, 2048), np.arange(1024, 1536),
    ])  # [i, f, o, g]

    gdbl = np.ones((2048, 1), np.float32)
    gdbl[1536:] = 2.0  # g-gates doubled: tanh(x) = 2*sigmoid(2x) - 1
    wihT = np.stack([_chunked_cols((Wih[l][perm] * gdbl).T) for l in range(2)])
    whhT = np.stack([_chunked_cols((Whh[l][perm] * gdbl).T) for l in range(2)])
    biascols = np.stack([
        ((bih[l] + bhh[l])[perm] * gdbl[:, 0]).reshape(NG, 128).T
        for l in range(2)
    ])

    st = np.zeros((128, NT * K), np.float32)
    for tt in range(NT):
        for p in range(128):
            st[p, tt * K + (tt * 128 + p) // CS] = 1.0

    winkeep = (np.arange(T)[None, :] >= (np.arange(K)[:, None] + 1) * CS)

    common = {
        "wihT": _f16(wihT), "whhT": _f16(whhT), "biascols": _f32(biascols),
        "ident": _bf(np.eye(128)), "identf": _f16(np.eye(128)), "sel": _bf(st),
        "wcpT": _bf(_chunked_cols(Wcp.T)), "bcpcols": _f32(bcp.reshape(4, 128).T),
        "wffT": _bf(_chunked_cols(Wff.T)), "bffcols": _f32(bff.reshape(4, 128).T),
        "wqT": _bf(_chunked_cols(Wq.T)), "bqcols": _f32(bq.reshape(4, 128).T),
        "wkT": _bf(_chunked_cols(Wkv[:FF].T)),
        "bkcols": _f32(bkv[:FF].reshape(4, 128).T),
        "wvT": _bf(_chunked_cols(Wkv[FF:].T)),
        "bvB": _f32(np.broadcast_to(bkv[FF:], (128, FF))),
        "woutT": _bf(_chunked_cols(Wout.T)),
        "boutB": _f32(np.broadcast_to(bout, (K, F))),
        "gamB": _f32(np.broadcast_to(gam, (128, H))),
        "betB": _f32(np.broadcast_to(bet, (128, H))),
        "epscol": _f32(np.full((128, 1), LN_EPS)),
        "zeros4": _f16(np.zeros((128, NC4))),
    }

    in_maps = []
    for b in range(B):
        m = dict(common)
        m["xT"] = _f16(_chunked_cols(x[b].T))
        m["noise"] = _f32(noise[b])
        m["Amask"] = _bf(winkeep * mask[b][None, :])
        m["Cmask"] = _bf(1.0 - mask[b][None, :] * np.ones((K, T), np.float32))
        in_maps.append(m)
    return in_maps


_NC_CACHE = {}


def kernel(**inputs):
    if "nc" not in _NC_CACHE:
        _NC_CACHE["nc"] = build_program()
    nc = _NC_CACHE["nc"]
    in_maps = prepare_in_maps(inputs)
    res = run_bass_kernel_spmd(nc, in_maps, core_ids=list(range(B)))
    ctx = np.stack([res.results[b]["out_ctx"] for b in range(B)])
    prb = np.stack([res.results[b]["out_probs"] for b in range(B)])
    return np.asarray(ctx, np.float32), np.asarray(prb, np.float32)
